# revision 17
# baseline (speedup 1.0000x reference)
"""Causal multi-head attention on 8 TRN2 NeuronCores.

Problem: inputs [4,2048,512] f32, attn_kernel [512,8,64,3], out_kernel
[64,8,64] -> out [4,2048,64] f32 (fused QKV, causal softmax, per-head AV,
head-summed output projection).

Sharding: core c -> (batch b=c//2, head-group hg=c%2 of 4 heads).  Each
core computes a partial output [2048,64] (sum over its 4 heads); the host
adds the two head-group partials per batch.

On-chip layout (per core), everything bf16 with f32 PSUM accumulation:
  xT [512,2048]   host-pretransposed input (i on partitions)
  Q^T,K^T [256,2048] = W^T xT  (partition-block pr holds heads 2pr,2pr+1)
  V [2048, 4, 65]  natural layout + a ones column (softmax denominator
                   falls out of the AV matmul as row 64)
  S^T[kb] = K_kb @ Q^T  ([128 keys, 512 queries]) -> exp -> P^T
  OH^T[h] [65, 512] accumulated over key blocks; row 64 = sum(exp)
  out = sum_h (OH_h/sums) @ Wo_h
Causality: for query chunk c (512 wide) only key blocks 0..4c+3 are
computed; the diagonal 512x512 region is masked by multiplying P^T with a
host-built [128,4,512] triangular mask (exp never overflows: |scores|<1).
"""
import math

import numpy as np
import ml_dtypes

B, N, D = 4, 2048, 512
HEADS, DH = 8, 64
NH = 4          # heads per core
NCH = 4         # query chunks of 512
CH = 512        # query chunk size
KB = 128        # key block size
OUT_D = 64

_BF16 = ml_dtypes.bfloat16
_built = None


def _install_tile_patch():
    """walrus rejects >N sync-waits on one instruction; Tile's exit drain
    aggregates one wait per live semaphore.  Split into 1-wait drains."""
    import concourse.tile as tile_mod
    from concourse.vector_clock import ScopedClock
    import concourse.mybir as mybir

    if getattr(tile_mod.TileContext, "_drain_patched", False):
        return

    def _drain_and_barrier(self, tick_clock, wait_clock):
        nc = self.nc
        drain_inst = nc.sync.drain()
        wait_clock.add_sem_waits(
            drain_inst.ins, ScopedClock({None: tick_clock.global_clock})
        )
        si = drain_inst.ins.sync_info
        if si is not None and len(si.on_wait) > 1:
            waits = list(si.on_wait)
            si.on_wait = waits[:1]
            for w in waits[1:]:
                extra = nc.sync.drain()
                ei = extra.ins
                if ei.sync_info is None:
                    ei.sync_info = mybir.SyncInfo(on_wait=[w], on_update=[])
                else:
                    ei.sync_info.on_wait = [w]
        nc.all_engine_barrier()
        assert self.sems is not None
        popped = nc._tile_sem_poison_stack.pop()
        assert popped is self._sem_poison
        nc.clear_and_free_semaphores(list(self.sems.allocated().values()))
        nc.all_engine_barrier()

    tile_mod.TileContext._drain_and_barrier = _drain_and_barrier
    tile_mod.TileContext._drain_patched = True


def _split_multi_waits(nc):
    """The TPB ISA takes one sync-wait per instruction; Tile's rust wait
    assigner sometimes attaches several.  Move extras onto injected
    same-engine NoOps immediately before the instruction."""
    import concourse.mybir as mybir

    cnt = 0
    for f in nc.m.functions:
        for b in f.blocks:
            new = []
            changed = False
            for inst in b.instructions:
                si = inst.sync_info
                if si is not None and len(si.on_wait) > 1:
                    waits = list(si.on_wait)
                    si.on_wait = waits[-1:]
                    for w in waits[:-1]:
                        nop = mybir.InstNoOp(
                            name=f"I-waitsplit-{cnt}", engine=inst.engine,
                            sync_info=mybir.SyncInfo(on_wait=[w], on_update=[]),
                            bass_nofuse=True)
                        cnt += 1
                        new.append(nop)
                    changed = True
                new.append(inst)
            if changed:
                b.instructions = new


def _build(split_waits=True):
    """Build the per-core Bass program (same SPMD graph for all cores)."""
    import concourse.bass as bass
    import concourse.mybir as mybir
    import concourse.tile as tile

    _install_tile_patch()
    f32 = mybir.dt.float32
    bf16 = mybir.dt.bfloat16
    Exp = mybir.ActivationFunctionType.Exp

    nc = bass.Bass()
    xt_ext = nc.declare_dram_parameter("xt", [D, N], bf16, isOutput=False)
    wq_ext = nc.declare_dram_parameter("wq", [D, NH * DH], bf16, isOutput=False)
    wk_ext = nc.declare_dram_parameter("wk", [D, NH * DH], bf16, isOutput=False)
    wv_ext = nc.declare_dram_parameter("wv", [D, NH * DH], bf16, isOutput=False)
    wo_ext = nc.declare_dram_parameter("wo", [DH, NH * OUT_D], bf16, isOutput=False)
    mk_ext = nc.declare_dram_parameter("mask", [KB, 4, CH], bf16, isOutput=False)
    out_ext = nc.declare_dram_parameter("out", [N, OUT_D], f32, isOutput=True)

    with tile.TileContext(nc) as tc:
        with tc.tile_pool(name="const", bufs=1) as const, \
             tc.tile_pool(name="persist", bufs=1) as persist, \
             tc.tile_pool(name="work", bufs=3) as work, \
             tc.tile_pool(name="ptp", bufs=4) as ptp, \
             tc.tile_pool(name="stp", bufs=2, space="PSUM") as stp, \
             tc.tile_pool(name="ohp", bufs=2, space="PSUM") as ohp, \
             tc.tile_pool(name="mixp", bufs=2, space="PSUM") as mixp, \
             tc.tile_pool(name="dramp", bufs=4, space="DRAM") as dramp:

            # ---- constants / weights ----
            xt = [const.tile([KB, N], bf16, tag=f"xt{i}", name=f"xt{i}") for i in range(4)]
            for i in range(4):
                nc.sync.dma_start(out=xt[i], in_=xt_ext[i * KB:(i + 1) * KB, :])
            wq = const.tile([KB, 4, NH * DH], bf16, tag="wq", name="wq")
            wk = const.tile([KB, 4, NH * DH], bf16, tag="wk", name="wk")
            wv = const.tile([KB, 4, NH * DH], bf16, tag="wv", name="wv")
            for i in range(4):
                nc.sync.dma_start(out=wq[:, i, :], in_=wq_ext[i * KB:(i + 1) * KB, :])
                nc.sync.dma_start(out=wk[:, i, :], in_=wk_ext[i * KB:(i + 1) * KB, :])
                nc.sync.dma_start(out=wv[:, i, :], in_=wv_ext[i * KB:(i + 1) * KB, :])
            wo = const.tile([DH, NH, OUT_D], bf16, tag="wo", name="wo")
            nc.sync.dma_start(
                out=wo, in_=wo_ext.rearrange("d (h o) -> d h o", h=NH))
            mask = const.tile([KB, 4, CH], bf16, tag="mask", name="mask")
            nc.sync.dma_start(out=mask, in_=mk_ext[:, :, :])

            # persistent per-chunk projections
            qt = [[persist.tile([KB, CH], bf16, tag=f"qt{p}{c}", name=f"qt{p}{c}") for c in range(NCH)]
                  for p in range(2)]
            kt = [[persist.tile([KB, CH], bf16, tag=f"kt{p}{c}", name=f"kt{p}{c}") for c in range(NCH)]
                  for p in range(2)]
            # V for chunk c: [128, nb(4), head(4), 65]
            vt = [persist.tile([KB, 4, NH, DH + 1], bf16, tag=f"vt{c}", name=f"vt{c}")
                  for c in range(NCH)]

            def project(c):
                for pr in range(2):
                    for dst, w in ((qt, wq), (kt, wk)):
                        pp = mixp.tile([KB, CH], f32, tag="mix", name="mix")
                        for ib in range(4):
                            nc.tensor.matmul(
                                pp,
                                lhsT=w[:, ib, pr * 128:(pr + 1) * 128],
                                rhs=xt[ib][:, c * CH:(c + 1) * CH],
                                start=(ib == 0), stop=(ib == 3))
                        nc.vector.tensor_copy(dst[pr][c], pp)
                v = vt[c]
                nc.vector.memset(v[:, :, :, DH:DH + 1], 1.0)
                for nb in range(4):
                    vp = mixp.tile([KB, NH * DH], f32, tag="mix", name="mix")
                    n0 = c * CH + nb * KB
                    for ib in range(4):
                        nc.tensor.matmul(
                            vp, lhsT=xt[ib][:, n0:n0 + KB], rhs=wv[:, ib, :],
                            start=(ib == 0), stop=(ib == 3))
                    nc.vector.tensor_copy(
                        v[:, nb, :, 0:DH],
                        vp.rearrange("p (h o) -> p h o", h=NH))

            def attend(c, pr):
                """Heads 2pr, 2pr+1 for query chunk c; returns ohn tiles."""
                ng = (c + 1) * 4          # key blocks
                oh = [ohp.tile([DH + 1, CH], f32, tag="oh", name="oh") for _ in range(2)]
                for kb in range(ng):
                    kc, ko = divmod(kb, 4)
                    diag = (kc == c)
                    o0 = ko * KB if diag else 0   # trimmed query range
                    st = stp.tile([KB, 2, CH], f32, tag="st", name="st")
                    for e in range(2):
                        nc.tensor.matmul(
                            st[:, e, o0:CH],
                            lhsT=kt[pr][kc][e * DH:(e + 1) * DH,
                                            ko * KB:(ko + 1) * KB],
                            rhs=qt[pr][c][e * DH:(e + 1) * DH, o0:CH],
                            start=True, stop=True)
                    pt = ptp.tile([KB, 2, CH], bf16, tag="pt", name="pt")
                    nc.scalar.activation(pt[:, :, o0:CH], st[:, :, o0:CH], Exp)
                    if diag:
                        for e in range(2):
                            nc.gpsimd.tensor_mul(
                                pt[:, e, o0:CH], pt[:, e, o0:CH],
                                mask[:, ko, o0:CH])
                    for e in range(2):
                        nc.tensor.matmul(
                            oh[e][:, o0:CH],
                            lhsT=vt[kc][:, ko, 2 * pr + e, :],
                            rhs=pt[:, e, o0:CH],
                            start=(kb == 0), stop=(kb == ng - 1),
                            skip_group_check=True)
                cat = work.tile([33, CH], f32, tag="cat", name="cat")
                for e in range(2):
                    nc.vector.tensor_copy(cat[32 * e:32 * e + 1, :],
                                          oh[e][DH:DH + 1, :])
                rec = work.tile([33, CH], f32, tag="rec", name="rec")
                nc.vector.reciprocal(rec[0:1, :], cat[0:1, :])
                nc.vector.reciprocal(rec[32:33, :], cat[32:33, :])
                rd = dramp.tile([2, CH], f32, tag="recd", name="recd")
                nc.sync.dma_start(out=rd[0:1, :], in_=rec[0:1, :])
                nc.sync.dma_start(out=rd[1:2, :], in_=rec[32:33, :])
                ohn = []
                for e in range(2):
                    rsl = rd[e:e + 1, :]
                    rb = bass.AP(tensor=rsl.tensor, offset=rsl.offset,
                                 ap=[[0, DH], rsl.ap[-1]])
                    rep = work.tile([DH, CH], f32, tag="rep", name="rep")
                    nc.sync.dma_start(out=rep, in_=rb)
                    on = work.tile([DH, CH], bf16, tag="ohn", name="ohn", bufs=8)
                    nc.vector.tensor_mul(on, oh[e][0:DH, :], rep)
                    ohn.append(on)
                return ohn

            for c in range(NCH):
                project(c)
                ohn4 = []
                for pr in range(2):
                    ohn4 += attend(c, pr)
                for nb in range(4):
                    op = mixp.tile([KB, OUT_D], f32, tag="mix", name="mix")
                    for h in range(NH):
                        nc.tensor.matmul(
                            op,
                            lhsT=ohn4[h][:, nb * KB:(nb + 1) * KB],
                            rhs=wo[:, h, :],
                            start=(h == 0), stop=(h == 3))
                    ob = work.tile([KB, OUT_D], f32, tag="ob", name="ob")
                    nc.vector.tensor_copy(ob, op)
                    r0 = c * CH + nb * KB
                    nc.sync.dma_start(out=out_ext[r0:r0 + KB, :], in_=ob)
    if split_waits:
        _split_multi_waits(nc)
    return nc


def _get_nc():
    global _built
    if _built is None:
        _built = _build()
    return _built


def _make_mask():
    p = np.arange(KB)[:, None, None]
    j = np.arange(4)[None, :, None]
    col = np.arange(CH)[None, None, :]
    return ((j * KB + p) <= col).astype(_BF16)


def _prepare_in_maps(inputs, attn_kernel, out_kernel):
    inputs = np.asarray(inputs, dtype=np.float32)
    attn_kernel = np.asarray(attn_kernel, dtype=np.float32)
    out_kernel = np.asarray(out_kernel, dtype=np.float32)
    mask = _make_mask()
    scale = 1.0 / math.sqrt(D)
    in_maps = []
    for c in range(8):
        b, hg = divmod(c, 2)
        hsl = slice(hg * NH, (hg + 1) * NH)
        in_maps.append({
            "xt": np.ascontiguousarray(inputs[b].T).astype(_BF16),
            "wq": np.ascontiguousarray(
                attn_kernel[:, hsl, :, 2].reshape(D, NH * DH) * scale).astype(_BF16),
            "wk": np.ascontiguousarray(
                attn_kernel[:, hsl, :, 0].reshape(D, NH * DH)).astype(_BF16),
            "wv": np.ascontiguousarray(
                attn_kernel[:, hsl, :, 1].reshape(D, NH * DH)).astype(_BF16),
            "wo": np.ascontiguousarray(
                out_kernel[:, hsl, :].reshape(DH, NH * OUT_D)).astype(_BF16),
            "mask": mask,
        })
    return in_maps


def _combine(results):
    out = np.empty((B, N, OUT_D), dtype=np.float32)
    for b in range(B):
        out[b] = results[2 * b]["out"] + results[2 * b + 1]["out"]
    return out


def _run(in_maps, trace=False, **kw):
    from concourse.bass_utils import run_bass_kernel_spmd
    nc = _get_nc()
    return run_bass_kernel_spmd(nc, in_maps, core_ids=list(range(8)),
                                trace=trace, **kw)


def kernel(inputs, attn_kernel, out_kernel):
    in_maps = _prepare_in_maps(inputs, attn_kernel, out_kernel)
    res = _run(in_maps, trace=False)
    return _combine(res.results)


# revision 18
# speedup vs baseline: 1.1233x; 1.1233x over previous
"""Causal multi-head attention on 8 TRN2 NeuronCores.

Problem: inputs [4,2048,512] f32, attn_kernel [512,8,64,3], out_kernel
[64,8,64] -> out [4,2048,64] f32 (fused QKV, causal softmax, per-head AV,
head-summed output projection).

Sharding: core c -> (batch b=c//2, head-group hg=c%2 of 4 heads).  Each
core computes a partial output [2048,64] (sum over its 4 heads); the host
adds the two head-group partials per batch.

On-chip layout (per core), everything bf16 with f32 PSUM accumulation:
  xT [512,2048]   host-pretransposed input (i on partitions)
  Q^T,K^T [256,2048] = W^T xT  (partition-block pr holds heads 2pr,2pr+1)
  V [2048, 4, 65]  natural layout + a ones column (softmax denominator
                   falls out of the AV matmul as row 64)
  S^T[kb] = K_kb @ Q^T  ([128 keys, 512 queries]) -> exp -> P^T
  OH^T[h] [65, 512] accumulated over key blocks; row 64 = sum(exp)
  out = sum_h (OH_h/sums) @ Wo_h
Causality: for query chunk c (512 wide) only key blocks 0..4c+3 are
computed; the diagonal 512x512 region is masked by multiplying P^T with a
host-built [128,4,512] triangular mask (exp never overflows: |scores|<1).
"""
import math

import numpy as np
import ml_dtypes

B, N, D = 4, 2048, 512
HEADS, DH = 8, 64
NH = 4          # heads per core
NCH = 4         # query chunks of 512
CH = 512        # query chunk size
KB = 128        # key block size
OUT_D = 64

_BF16 = ml_dtypes.bfloat16
_built = None


def _install_tile_patch():
    """walrus rejects >N sync-waits on one instruction; Tile's exit drain
    aggregates one wait per live semaphore.  Split into 1-wait drains."""
    import concourse.tile as tile_mod
    from concourse.vector_clock import ScopedClock
    import concourse.mybir as mybir

    if getattr(tile_mod.TileContext, "_drain_patched", False):
        return

    def _drain_and_barrier(self, tick_clock, wait_clock):
        nc = self.nc
        drain_inst = nc.sync.drain()
        wait_clock.add_sem_waits(
            drain_inst.ins, ScopedClock({None: tick_clock.global_clock})
        )
        si = drain_inst.ins.sync_info
        if si is not None and len(si.on_wait) > 1:
            waits = list(si.on_wait)
            si.on_wait = waits[:1]
            for w in waits[1:]:
                extra = nc.sync.drain()
                ei = extra.ins
                if ei.sync_info is None:
                    ei.sync_info = mybir.SyncInfo(on_wait=[w], on_update=[])
                else:
                    ei.sync_info.on_wait = [w]
        nc.all_engine_barrier()
        assert self.sems is not None
        popped = nc._tile_sem_poison_stack.pop()
        assert popped is self._sem_poison
        nc.clear_and_free_semaphores(list(self.sems.allocated().values()))
        nc.all_engine_barrier()

    tile_mod.TileContext._drain_and_barrier = _drain_and_barrier
    tile_mod.TileContext._drain_patched = True


def _split_multi_waits(nc):
    """The TPB ISA takes one sync-wait per instruction; Tile's rust wait
    assigner sometimes attaches several.  Move extras onto injected
    same-engine NoOps immediately before the instruction."""
    import concourse.mybir as mybir

    cnt = 0
    for f in nc.m.functions:
        for b in f.blocks:
            new = []
            changed = False
            for inst in b.instructions:
                si = inst.sync_info
                if si is not None and len(si.on_wait) > 1:
                    waits = list(si.on_wait)
                    si.on_wait = waits[-1:]
                    for w in waits[:-1]:
                        nop = mybir.InstNoOp(
                            name=f"I-waitsplit-{cnt}", engine=inst.engine,
                            sync_info=mybir.SyncInfo(on_wait=[w], on_update=[]),
                            bass_nofuse=True)
                        cnt += 1
                        new.append(nop)
                    changed = True
                new.append(inst)
            if changed:
                b.instructions = new


def _build(split_waits=True):
    """Build the per-core Bass program (same SPMD graph for all cores)."""
    import concourse.bass as bass
    import concourse.mybir as mybir
    import concourse.tile as tile

    _install_tile_patch()
    f32 = mybir.dt.float32
    bf16 = mybir.dt.bfloat16
    Exp = mybir.ActivationFunctionType.Exp

    nc = bass.Bass()
    xt_ext = nc.declare_dram_parameter("xt", [D, N], bf16, isOutput=False)
    wq_ext = nc.declare_dram_parameter("wq", [D, NH * DH], bf16, isOutput=False)
    wk_ext = nc.declare_dram_parameter("wk", [D, NH * DH], bf16, isOutput=False)
    wv_ext = nc.declare_dram_parameter("wv", [D, NH * DH], bf16, isOutput=False)
    wo_ext = nc.declare_dram_parameter("wo", [DH, NH * OUT_D], bf16, isOutput=False)
    mk_ext = nc.declare_dram_parameter("mask", [KB, 4, CH], bf16, isOutput=False)
    out_ext = nc.declare_dram_parameter("out", [N, OUT_D], f32, isOutput=True)

    with tile.TileContext(nc) as tc:
        with tc.tile_pool(name="const", bufs=1) as const, \
             tc.tile_pool(name="persist", bufs=1) as persist, \
             tc.tile_pool(name="work", bufs=3) as work, \
             tc.tile_pool(name="ptp", bufs=4) as ptp, \
             tc.tile_pool(name="stp", bufs=2, space="PSUM") as stp, \
             tc.tile_pool(name="ohp", bufs=2, space="PSUM") as ohp, \
             tc.tile_pool(name="mixp", bufs=2, space="PSUM") as mixp, \
             tc.tile_pool(name="dramp", bufs=4, space="DRAM") as dramp:

            # ---- constants / weights ----
            xt = [const.tile([KB, N], bf16, tag=f"xt{i}", name=f"xt{i}") for i in range(4)]
            for i in range(4):
                nc.sync.dma_start(out=xt[i], in_=xt_ext[i * KB:(i + 1) * KB, :])
            wq = const.tile([KB, 4, NH * DH], bf16, tag="wq", name="wq")
            wk = const.tile([KB, 4, NH * DH], bf16, tag="wk", name="wk")
            wv = const.tile([KB, 4, NH * DH], bf16, tag="wv", name="wv")
            for i in range(4):
                nc.sync.dma_start(out=wq[:, i, :], in_=wq_ext[i * KB:(i + 1) * KB, :])
                nc.sync.dma_start(out=wk[:, i, :], in_=wk_ext[i * KB:(i + 1) * KB, :])
                nc.sync.dma_start(out=wv[:, i, :], in_=wv_ext[i * KB:(i + 1) * KB, :])
            wo = const.tile([DH, NH, OUT_D], bf16, tag="wo", name="wo")
            nc.sync.dma_start(
                out=wo, in_=wo_ext.rearrange("d (h o) -> d h o", h=NH))
            mask = const.tile([KB, 4, CH], bf16, tag="mask", name="mask")
            nc.sync.dma_start(out=mask, in_=mk_ext[:, :, :])

            # persistent per-chunk projections
            qt = [[persist.tile([KB, CH], bf16, tag=f"qt{p}{c}", name=f"qt{p}{c}") for c in range(NCH)]
                  for p in range(2)]
            kt = [[persist.tile([KB, CH], bf16, tag=f"kt{p}{c}", name=f"kt{p}{c}") for c in range(NCH)]
                  for p in range(2)]
            # V for chunk c: [128, nb(4), head(4), 65]
            vt = [persist.tile([KB, 4, NH, DH + 1], bf16, tag=f"vt{c}", name=f"vt{c}")
                  for c in range(NCH)]

            def project(c):
                for pr in range(2):
                    for dst, w in ((qt, wq), (kt, wk)):
                        pp = mixp.tile([KB, CH], f32, tag="mix", name="mix")
                        for ib in range(4):
                            nc.tensor.matmul(
                                pp,
                                lhsT=w[:, ib, pr * 128:(pr + 1) * 128],
                                rhs=xt[ib][:, c * CH:(c + 1) * CH],
                                start=(ib == 0), stop=(ib == 3))
                        nc.vector.tensor_copy(dst[pr][c], pp)
                v = vt[c]
                nc.vector.memset(v[:, :, :, DH:DH + 1], 1.0)
                for nb in range(4):
                    vp = mixp.tile([KB, NH * DH], f32, tag="mix", name="mix")
                    n0 = c * CH + nb * KB
                    for ib in range(4):
                        nc.tensor.matmul(
                            vp, lhsT=xt[ib][:, n0:n0 + KB], rhs=wv[:, ib, :],
                            start=(ib == 0), stop=(ib == 3))
                    nc.vector.tensor_copy(
                        v[:, nb, :, 0:DH],
                        vp.rearrange("p (h o) -> p h o", h=NH))

            def attend(c, pr):
                """Heads 2pr, 2pr+1 for query chunk c; returns ohn tiles."""
                ng = (c + 1) * 4          # key blocks
                oh = [ohp.tile([DH + 1, CH], f32, tag="oh", name="oh") for _ in range(2)]
                for kb in range(ng):
                    kc, ko = divmod(kb, 4)
                    diag = (kc == c)
                    o0 = ko * KB if diag else 0   # trimmed query range
                    st = stp.tile([KB, 2, CH], f32, tag="st", name="st")
                    for e in range(2):
                        nc.tensor.matmul(
                            st[:, e, o0:CH],
                            lhsT=kt[pr][kc][e * DH:(e + 1) * DH,
                                            ko * KB:(ko + 1) * KB],
                            rhs=qt[pr][c][e * DH:(e + 1) * DH, o0:CH],
                            start=True, stop=True)
                    pt = ptp.tile([KB, 2, CH], bf16, tag="pt", name="pt")
                    nc.scalar.activation(pt[:, :, o0:CH], st[:, :, o0:CH], Exp)
                    if diag:
                        for e in range(2):
                            nc.gpsimd.tensor_mul(
                                pt[:, e, o0:CH], pt[:, e, o0:CH],
                                mask[:, ko, o0:CH])
                    for e in range(2):
                        nc.tensor.matmul(
                            oh[e][:, o0:CH],
                            lhsT=vt[kc][:, ko, 2 * pr + e, :],
                            rhs=pt[:, e, o0:CH],
                            start=(kb == 0), stop=(kb == ng - 1),
                            skip_group_check=True)
                # drain PSUM fast: unnormalized OH + sums rows -> SBUF
                us, cat = [], work.tile([33, CH], f32, tag="cat", name="cat")
                for e in range(2):
                    u = work.tile([DH, CH], bf16, tag="u", name="u", bufs=6)
                    nc.vector.tensor_copy(u, oh[e][0:DH, :])
                    us.append(u)
                    nc.vector.tensor_copy(cat[32 * e:32 * e + 1, :],
                                          oh[e][DH:DH + 1, :])
                # reshape sums through DRAM so reciprocal is 8 cols, not 512
                rd = dramp.tile([2, CH], f32, tag="recd", name="recd")
                nc.sync.dma_start(out=rd[0:1, :], in_=cat[0:1, :])
                nc.sync.dma_start(out=rd[1:2, :], in_=cat[32:33, :])
                rsh = bass.AP(tensor=rd.tensor, offset=rd.offset,
                              ap=[[4, KB], [CH, 2], [1, 4]])
                rc = work.tile([KB, 2, 4], f32, tag="rc", name="rc")
                nc.sync.dma_start(out=rc, in_=rsh)
                nc.vector.reciprocal(rc, rc)
                rd2 = dramp.tile([2, CH], f32, tag="recd2", name="recd2")
                rsh2 = bass.AP(tensor=rd2.tensor, offset=rd2.offset,
                               ap=[[4, KB], [CH, 2], [1, 4]])
                nc.sync.dma_start(out=rsh2, in_=rc)
                ohn = []
                for e in range(2):
                    rsl = rd2[e:e + 1, :]
                    rb = bass.AP(tensor=rsl.tensor, offset=rsl.offset,
                                 ap=[[0, DH], rsl.ap[-1]])
                    rep = work.tile([DH, CH], f32, tag="rep", name="rep")
                    nc.sync.dma_start(out=rep, in_=rb)
                    on = work.tile([DH, CH], bf16, tag="ohn", name="ohn", bufs=8)
                    nc.vector.tensor_mul(on, us[e], rep)
                    ohn.append(on)
                return ohn

            for c in range(NCH):
                project(c)
                ohn4 = []
                for pr in range(2):
                    ohn4 += attend(c, pr)
                for nb in range(4):
                    op = mixp.tile([KB, OUT_D], f32, tag="mix", name="mix")
                    for h in range(NH):
                        nc.tensor.matmul(
                            op,
                            lhsT=ohn4[h][:, nb * KB:(nb + 1) * KB],
                            rhs=wo[:, h, :],
                            start=(h == 0), stop=(h == 3))
                    ob = work.tile([KB, OUT_D], f32, tag="ob", name="ob")
                    nc.vector.tensor_copy(ob, op)
                    r0 = c * CH + nb * KB
                    nc.sync.dma_start(out=out_ext[r0:r0 + KB, :], in_=ob)
    if split_waits:
        _split_multi_waits(nc)
    return nc


def _get_nc():
    global _built
    if _built is None:
        _built = _build()
    return _built


def _make_mask():
    p = np.arange(KB)[:, None, None]
    j = np.arange(4)[None, :, None]
    col = np.arange(CH)[None, None, :]
    return ((j * KB + p) <= col).astype(_BF16)


def _prepare_in_maps(inputs, attn_kernel, out_kernel):
    inputs = np.asarray(inputs, dtype=np.float32)
    attn_kernel = np.asarray(attn_kernel, dtype=np.float32)
    out_kernel = np.asarray(out_kernel, dtype=np.float32)
    mask = _make_mask()
    scale = 1.0 / math.sqrt(D)
    in_maps = []
    for c in range(8):
        b, hg = divmod(c, 2)
        hsl = slice(hg * NH, (hg + 1) * NH)
        in_maps.append({
            "xt": np.ascontiguousarray(inputs[b].T).astype(_BF16),
            "wq": np.ascontiguousarray(
                attn_kernel[:, hsl, :, 2].reshape(D, NH * DH) * scale).astype(_BF16),
            "wk": np.ascontiguousarray(
                attn_kernel[:, hsl, :, 0].reshape(D, NH * DH)).astype(_BF16),
            "wv": np.ascontiguousarray(
                attn_kernel[:, hsl, :, 1].reshape(D, NH * DH)).astype(_BF16),
            "wo": np.ascontiguousarray(
                out_kernel[:, hsl, :].reshape(DH, NH * OUT_D)).astype(_BF16),
            "mask": mask,
        })
    return in_maps


def _combine(results):
    out = np.empty((B, N, OUT_D), dtype=np.float32)
    for b in range(B):
        out[b] = results[2 * b]["out"] + results[2 * b + 1]["out"]
    return out


def _run(in_maps, trace=False, **kw):
    from concourse.bass_utils import run_bass_kernel_spmd
    nc = _get_nc()
    return run_bass_kernel_spmd(nc, in_maps, core_ids=list(range(8)),
                                trace=trace, **kw)


def kernel(inputs, attn_kernel, out_kernel):
    in_maps = _prepare_in_maps(inputs, attn_kernel, out_kernel)
    res = _run(in_maps, trace=False)
    return _combine(res.results)


# revision 19
# speedup vs baseline: 1.4448x; 1.2862x over previous
"""Causal multi-head attention on 8 TRN2 NeuronCores.

Problem: inputs [4,2048,512] f32, attn_kernel [512,8,64,3], out_kernel
[64,8,64] -> out [4,2048,64] f32 (fused QKV, causal softmax, per-head AV,
head-summed output projection).

Sharding: core c -> (batch b=c//2, head-group hg=c%2 of 4 heads).  Each
core computes a partial output [2048,64] (sum over its 4 heads); the host
adds the two head-group partials per batch.

On-chip layout (per core), everything bf16 with f32 PSUM accumulation:
  xT [512,2048]   host-pretransposed input (i on partitions)
  Q^T,K^T [256,2048] = W^T xT  (partition-block pr holds heads 2pr,2pr+1)
  V [2048, 4, 65]  natural layout + a ones column (softmax denominator
                   falls out of the AV matmul as row 64)
  S^T[kb] = K_kb @ Q^T  ([128 keys, 512 queries]) -> exp -> P^T
  OH^T[h] [65, 512] accumulated over key blocks; row 64 = sum(exp)
  out = sum_h (OH_h/sums) @ Wo_h
Causality: for query chunk c (512 wide) only key blocks 0..4c+3 are
computed; the diagonal 512x512 region is masked by multiplying P^T with a
host-built [128,4,512] triangular mask (exp never overflows: |scores|<1).
"""
import math

import numpy as np
import ml_dtypes

B, N, D = 4, 2048, 512
HEADS, DH = 8, 64
NH = 4          # heads per core
NCH = 4         # query chunks of 512
CH = 512        # query chunk size
KB = 128        # key block size
OUT_D = 64

_BF16 = ml_dtypes.bfloat16
_built = None


def _install_tile_patch():
    """walrus rejects >N sync-waits on one instruction; Tile's exit drain
    aggregates one wait per live semaphore.  Split into 1-wait drains."""
    import concourse.tile as tile_mod
    from concourse.vector_clock import ScopedClock
    import concourse.mybir as mybir

    if getattr(tile_mod.TileContext, "_drain_patched", False):
        return

    def _drain_and_barrier(self, tick_clock, wait_clock):
        nc = self.nc
        drain_inst = nc.sync.drain()
        wait_clock.add_sem_waits(
            drain_inst.ins, ScopedClock({None: tick_clock.global_clock})
        )
        si = drain_inst.ins.sync_info
        if si is not None and len(si.on_wait) > 1:
            waits = list(si.on_wait)
            si.on_wait = waits[:1]
            for w in waits[1:]:
                extra = nc.sync.drain()
                ei = extra.ins
                if ei.sync_info is None:
                    ei.sync_info = mybir.SyncInfo(on_wait=[w], on_update=[])
                else:
                    ei.sync_info.on_wait = [w]
        nc.all_engine_barrier()
        assert self.sems is not None
        popped = nc._tile_sem_poison_stack.pop()
        assert popped is self._sem_poison
        nc.clear_and_free_semaphores(list(self.sems.allocated().values()))
        nc.all_engine_barrier()

    tile_mod.TileContext._drain_and_barrier = _drain_and_barrier
    tile_mod.TileContext._drain_patched = True


def _split_multi_waits(nc):
    """The TPB ISA takes one sync-wait per instruction; Tile's rust wait
    assigner sometimes attaches several.  Move extras onto injected
    same-engine NoOps immediately before the instruction."""
    import concourse.mybir as mybir

    cnt = 0
    for f in nc.m.functions:
        for b in f.blocks:
            new = []
            changed = False
            for inst in b.instructions:
                si = inst.sync_info
                if si is not None and len(si.on_wait) > 1:
                    waits = list(si.on_wait)
                    si.on_wait = waits[-1:]
                    for w in waits[:-1]:
                        nop = mybir.InstNoOp(
                            name=f"I-waitsplit-{cnt}", engine=inst.engine,
                            sync_info=mybir.SyncInfo(on_wait=[w], on_update=[]),
                            bass_nofuse=True)
                        cnt += 1
                        new.append(nop)
                    changed = True
                new.append(inst)
            if changed:
                b.instructions = new


def _build(split_waits=True):
    """Build the per-core Bass program (same SPMD graph for all cores)."""
    import concourse.bass as bass
    import concourse.mybir as mybir
    import concourse.tile as tile

    _install_tile_patch()
    f32 = mybir.dt.float32
    bf16 = mybir.dt.bfloat16
    Exp = mybir.ActivationFunctionType.Exp

    nc = bass.Bass()
    xt_ext = nc.declare_dram_parameter("xt", [D, N], bf16, isOutput=False)
    wq_ext = nc.declare_dram_parameter("wq", [D, NH * DH], bf16, isOutput=False)
    wk_ext = nc.declare_dram_parameter("wk", [D, NH * DH], bf16, isOutput=False)
    wv_ext = nc.declare_dram_parameter("wv", [D, NH * DH], bf16, isOutput=False)
    wo_ext = nc.declare_dram_parameter("wo", [DH, NH * OUT_D], bf16, isOutput=False)
    mk_ext = nc.declare_dram_parameter("mask", [KB, 4, CH], bf16, isOutput=False)
    out_ext = nc.declare_dram_parameter("out", [N, OUT_D], f32, isOutput=True)

    with tile.TileContext(nc) as tc:
        with tc.tile_pool(name="const", bufs=1) as const, \
             tc.tile_pool(name="persist", bufs=1) as persist, \
             tc.tile_pool(name="work", bufs=3) as work, \
             tc.tile_pool(name="ptp", bufs=4) as ptp, \
             tc.tile_pool(name="stp", bufs=2, space="PSUM") as stp, \
             tc.tile_pool(name="ohp", bufs=2, space="PSUM") as ohp, \
             tc.tile_pool(name="mixp", bufs=2, space="PSUM") as mixp, \
             tc.tile_pool(name="dramp", bufs=4, space="DRAM") as dramp:

            # ---- constants / weights ----
            xt = [const.tile([KB, N], bf16, tag=f"xt{i}", name=f"xt{i}") for i in range(4)]
            for i in range(4):
                nc.sync.dma_start(out=xt[i], in_=xt_ext[i * KB:(i + 1) * KB, :])
            wq = const.tile([KB, 4, NH * DH], bf16, tag="wq", name="wq")
            wk = const.tile([KB, 4, NH * DH], bf16, tag="wk", name="wk")
            wv = const.tile([KB, 4, NH * DH], bf16, tag="wv", name="wv")
            for i in range(4):
                nc.sync.dma_start(out=wq[:, i, :], in_=wq_ext[i * KB:(i + 1) * KB, :])
                nc.sync.dma_start(out=wk[:, i, :], in_=wk_ext[i * KB:(i + 1) * KB, :])
                nc.sync.dma_start(out=wv[:, i, :], in_=wv_ext[i * KB:(i + 1) * KB, :])
            wo = const.tile([DH, NH, OUT_D], bf16, tag="wo", name="wo")
            nc.sync.dma_start(
                out=wo, in_=wo_ext.rearrange("d (h o) -> d h o", h=NH))
            mask = const.tile([KB, 4, CH], bf16, tag="mask", name="mask")
            nc.sync.dma_start(out=mask, in_=mk_ext[:, :, :])

            # persistent per-chunk projections
            qt = [[persist.tile([KB, CH], bf16, tag=f"qt{p}{c}", name=f"qt{p}{c}") for c in range(NCH)]
                  for p in range(2)]
            kt = [[persist.tile([KB, CH], bf16, tag=f"kt{p}{c}", name=f"kt{p}{c}") for c in range(NCH)]
                  for p in range(2)]
            # V for chunk c: [128, nb(4), head(4), 65]
            vt = [persist.tile([KB, 4, NH, DH + 1], bf16, tag=f"vt{c}", name=f"vt{c}")
                  for c in range(NCH)]

            def project(c):
                for pr in range(2):
                    for dst, w in ((qt, wq), (kt, wk)):
                        pp = mixp.tile([KB, CH], f32, tag="mix", name="mix")
                        for ib in range(4):
                            nc.tensor.matmul(
                                pp,
                                lhsT=w[:, ib, pr * 128:(pr + 1) * 128],
                                rhs=xt[ib][:, c * CH:(c + 1) * CH],
                                start=(ib == 0), stop=(ib == 3))
                        nc.vector.tensor_copy(dst[pr][c], pp)
                v = vt[c]
                nc.vector.memset(v[:, :, :, DH:DH + 1], 1.0)
                for nb in range(4):
                    vp = mixp.tile([KB, NH * DH], f32, tag="mix", name="mix")
                    n0 = c * CH + nb * KB
                    for ib in range(4):
                        nc.tensor.matmul(
                            vp, lhsT=xt[ib][:, n0:n0 + KB], rhs=wv[:, ib, :],
                            start=(ib == 0), stop=(ib == 3))
                    nc.vector.tensor_copy(
                        v[:, nb, :, 0:DH],
                        vp.rearrange("p (h o) -> p h o", h=NH))

            def attend(c, pr):
                """Heads 2pr, 2pr+1 for query chunk c; returns ohn tiles."""
                ng = (c + 1) * 4          # key blocks
                oh = [ohp.tile([DH + 1, CH], f32, tag="oh", name="oh") for _ in range(2)]
                for kb in range(ng):
                    kc, ko = divmod(kb, 4)
                    diag = (kc == c)
                    o0 = ko * KB if diag else 0   # trimmed query range
                    st = stp.tile([KB, 2, CH], f32, tag="st", name="st")
                    for e in range(2):
                        nc.tensor.matmul(
                            st[:, e, o0:CH],
                            lhsT=kt[pr][kc][e * DH:(e + 1) * DH,
                                            ko * KB:(ko + 1) * KB],
                            rhs=qt[pr][c][e * DH:(e + 1) * DH, o0:CH],
                            start=True, stop=True)
                    pt = ptp.tile([KB, 2, CH], bf16, tag="pt", name="pt")
                    nc.scalar.activation(pt[:, :, o0:CH], st[:, :, o0:CH], Exp)
                    if diag:
                        for e in range(2):
                            nc.gpsimd.tensor_mul(
                                pt[:, e, o0:CH], pt[:, e, o0:CH],
                                mask[:, ko, o0:CH])
                    for e in range(2):
                        nc.tensor.matmul(
                            oh[e][:, o0:CH],
                            lhsT=vt[kc][:, ko, 2 * pr + e, :],
                            rhs=pt[:, e, o0:CH],
                            start=(kb == 0), stop=(kb == ng - 1),
                            skip_group_check=True)
                # drain PSUM fast: unnormalized OH + sums rows -> SBUF
                us, cat = [], work.tile([33, CH], f32, tag="cat", name="cat")
                for e in range(2):
                    u = work.tile([DH, CH], bf16, tag="u", name="u", bufs=6)
                    nc.vector.tensor_copy(u, oh[e][0:DH, :])
                    us.append(u)
                    nc.vector.tensor_copy(cat[32 * e:32 * e + 1, :],
                                          oh[e][DH:DH + 1, :])
                # reshape sums through DRAM so reciprocal is 8 cols, not 512
                rd = dramp.tile([2, CH], f32, tag="recd", name="recd")
                nc.sync.dma_start(out=rd[0:1, :], in_=cat[0:1, :])
                nc.sync.dma_start(out=rd[1:2, :], in_=cat[32:33, :])
                rsh = bass.AP(tensor=rd.tensor, offset=rd.offset,
                              ap=[[4, KB], [CH, 2], [1, 4]])
                rc = work.tile([KB, 2, 4], f32, tag="rc", name="rc")
                nc.sync.dma_start(out=rc, in_=rsh)
                nc.vector.reciprocal(rc, rc)
                rd2 = dramp.tile([2, CH], f32, tag="recd2", name="recd2")
                rsh2 = bass.AP(tensor=rd2.tensor, offset=rd2.offset,
                               ap=[[4, KB], [CH, 2], [1, 4]])
                nc.sync.dma_start(out=rsh2, in_=rc)
                ohn = []
                for e in range(2):
                    rsl = rd2[e:e + 1, :]
                    rb = bass.AP(tensor=rsl.tensor, offset=rsl.offset,
                                 ap=[[0, DH], rsl.ap[-1]])
                    rep = work.tile([DH, CH], f32, tag="rep", name="rep")
                    nc.sync.dma_start(out=rep, in_=rb)
                    on = work.tile([DH, CH], bf16, tag="ohn", name="ohn", bufs=8)
                    nc.vector.tensor_mul(on, us[e], rep)
                    ohn.append(on)
                return ohn

            project(0)
            for c in range(NCH):
                ohn4 = []
                ohn4 += attend(c, 0)
                if c + 1 < NCH:
                    project(c + 1)
                ohn4 += attend(c, 1)
                for nb in range(4):
                    op = mixp.tile([KB, OUT_D], f32, tag="mix", name="mix")
                    for h in range(NH):
                        nc.tensor.matmul(
                            op,
                            lhsT=ohn4[h][:, nb * KB:(nb + 1) * KB],
                            rhs=wo[:, h, :],
                            start=(h == 0), stop=(h == 3))
                    ob = work.tile([KB, OUT_D], f32, tag="ob", name="ob")
                    nc.vector.tensor_copy(ob, op)
                    r0 = c * CH + nb * KB
                    nc.sync.dma_start(out=out_ext[r0:r0 + KB, :], in_=ob)
    if split_waits:
        _split_multi_waits(nc)
    return nc


def _get_nc():
    global _built
    if _built is None:
        _built = _build()
    return _built


def _make_mask():
    p = np.arange(KB)[:, None, None]
    j = np.arange(4)[None, :, None]
    col = np.arange(CH)[None, None, :]
    return ((j * KB + p) <= col).astype(_BF16)


def _prepare_in_maps(inputs, attn_kernel, out_kernel):
    inputs = np.asarray(inputs, dtype=np.float32)
    attn_kernel = np.asarray(attn_kernel, dtype=np.float32)
    out_kernel = np.asarray(out_kernel, dtype=np.float32)
    mask = _make_mask()
    scale = 1.0 / math.sqrt(D)
    in_maps = []
    for c in range(8):
        b, hg = divmod(c, 2)
        hsl = slice(hg * NH, (hg + 1) * NH)
        in_maps.append({
            "xt": np.ascontiguousarray(inputs[b].T).astype(_BF16),
            "wq": np.ascontiguousarray(
                attn_kernel[:, hsl, :, 2].reshape(D, NH * DH) * scale).astype(_BF16),
            "wk": np.ascontiguousarray(
                attn_kernel[:, hsl, :, 0].reshape(D, NH * DH)).astype(_BF16),
            "wv": np.ascontiguousarray(
                attn_kernel[:, hsl, :, 1].reshape(D, NH * DH)).astype(_BF16),
            "wo": np.ascontiguousarray(
                out_kernel[:, hsl, :].reshape(DH, NH * OUT_D)).astype(_BF16),
            "mask": mask,
        })
    return in_maps


def _combine(results):
    out = np.empty((B, N, OUT_D), dtype=np.float32)
    for b in range(B):
        out[b] = results[2 * b]["out"] + results[2 * b + 1]["out"]
    return out


def _run(in_maps, trace=False, **kw):
    from concourse.bass_utils import run_bass_kernel_spmd
    nc = _get_nc()
    return run_bass_kernel_spmd(nc, in_maps, core_ids=list(range(8)),
                                trace=trace, **kw)


def kernel(inputs, attn_kernel, out_kernel):
    in_maps = _prepare_in_maps(inputs, attn_kernel, out_kernel)
    res = _run(in_maps, trace=False)
    return _combine(res.results)
